# revision 4
# baseline (speedup 1.0000x reference)
"""Debiased EMA kernel (v5): bf16 device I/O + sliding-window two-matmul.

out_block_i = P.T @ x_block_{i-1} + C.T @ x_block_i in 128-row blocks;
debias folded into block-0/1 weights.  Host casts x to bf16 and
pre-permutes to a [128, nblk*C] tiled layout so every DMA is a fully
contiguous 8KB-per-partition transfer; device computes bf16 matmuls
(1 cycle/row vs fp32's 4) into fp32 PSUM and writes bf16, host upcasts.
Halves HBM traffic both ways: ~94us DMA floor vs 187us for fp32.
"""

import sys

for _p in ("/opt/trn_rl_repo", "/opt/pypackages"):
    if _p not in sys.path:
        sys.path.insert(0, _p)

import numpy as np
import ml_dtypes

import concourse.bacc as bacc
import concourse.mybir as mybir
from concourse import bass_utils
from concourse.tile import TileContext

B, T, C = 32, 4096, 512
NCORES = 8
BPC = B // NCORES
L = 128
NBLK = T // L
ALPHA = 0.9
DENOM_MIN = 1e-6

F32 = mybir.dt.float32
BF16 = mybir.dt.bfloat16
NPBF16 = ml_dtypes.bfloat16


def _build_weights() -> np.ndarray:
    a = float(np.float32(ALPHA))
    omb = 1.0 - a
    k = np.arange(L, dtype=np.float64)[:, None]
    m = np.arange(L, dtype=np.float64)[None, :]
    tri = (m - k) >= 0
    t = np.arange(2 * L, dtype=np.float64)
    d = np.maximum(1.0 - a ** (t + 1.0), DENOM_MIN)
    dec = np.where(tri, a ** np.where(tri, m - k, 0.0), 0.0)
    x0col = (k == 0)
    A0 = np.where(tri, np.where(x0col, a**m, omb * dec), 0.0) / d[:L][None, :]
    P1 = np.where(x0col, a ** (128.0 + m), omb * a ** (128.0 + m - k)) \
        / d[L:][None, :]
    C1 = omb * dec / d[L:][None, :]
    P = omb * a ** (128.0 + m - k)
    Cm = omb * dec
    w = np.concatenate([A0, P1, C1, P, Cm], axis=1)
    return np.ascontiguousarray(w.astype(NPBF16))


def build_program(bpc: int = BPC, t_len: int = T, chunk: int = 4):
    nblk = t_len // L
    nchunk = nblk // chunk
    assert nblk * L == t_len and nchunk * chunk == nblk

    nc = bacc.Bacc("TRN2", target_bir_lowering=False, debug=False)
    x = nc.dram_tensor("x", [bpc * L, nblk * C], BF16, kind="ExternalInput").ap()
    w = nc.dram_tensor("w", [L, 5 * L], BF16, kind="ExternalInput").ap()
    y = nc.dram_tensor("y", [bpc * L, nblk * C], BF16, kind="ExternalOutput").ap()

    with TileContext(nc) as tc:
        with (
            tc.tile_pool(name="wpool", bufs=1) as wpool,
            tc.tile_pool(name="xpool", bufs=8) as xpool,
            tc.tile_pool(name="ypool", bufs=8) as ypool,
            tc.tile_pool(name="psum", bufs=8, space="PSUM") as ppool,
        ):
            # Discarded matmuls on a memset tile: ramps the PE clock (HAM
            # gate) to full speed during the preamble, before the weight
            # DMA even lands.
            wsrc = wpool.tile([L, C], BF16, name="warm_src")
            nc.vector.memset(wsrc[:, :], 0.0)
            warm = ppool.tile([L, C], F32, tag="ps", name="warm_ps")
            for _ in range(12):
                nc.tensor.matmul(warm[:, :], wsrc[:, 0:L], wsrc[:, :],
                                 start=True, stop=True)
            wt = wpool.tile([L, 5 * L], BF16)
            nc.sync.dma_start(out=wt[:, :], in_=w[:, :])
            A0w = wt[:, 0 * L:1 * L]
            P1w = wt[:, 1 * L:2 * L]
            C1w = wt[:, 2 * L:3 * L]
            Pw = wt[:, 3 * L:4 * L]
            Cw = wt[:, 4 * L:5 * L]

            eng_i = 0
            for b in range(bpc):
                prev_chunk = None
                for ch in range(nchunk):
                    c0 = ch * chunk * C
                    xt = xpool.tile([L, chunk * C], BF16, tag="xt",
                                    name=f"xt_{b}_{ch}")
                    nc.sync.dma_start(
                        out=xt[:, :],
                        in_=x[b * L:(b + 1) * L, c0:c0 + chunk * C])
                    yt = ypool.tile([L, chunk * C], BF16, tag="yt",
                                    name=f"yt_{b}_{ch}")
                    for j in range(chunk):
                        i = ch * chunk + j
                        cur = xt[:, j * C:(j + 1) * C]
                        ps = ppool.tile([L, C], F32, tag="ps",
                                        name=f"ps_{b}_{ch}_{j}")
                        if i == 0:
                            nc.tensor.matmul(ps[:, :], A0w, cur,
                                             start=True, stop=True)
                        else:
                            prev = (xt[:, (j - 1) * C:j * C] if j > 0
                                    else prev_chunk[:, (chunk - 1) * C:])
                            pw, cw = (P1w, C1w) if i == 1 else (Pw, Cw)
                            nc.tensor.matmul(ps[:, :], pw, prev,
                                             start=True, stop=False)
                            nc.tensor.matmul(ps[:, :], cw, cur,
                                             start=False, stop=True)
                        dst = yt[:, j * C:(j + 1) * C]
                        if eng_i % 2 == 0:
                            nc.vector.tensor_copy(out=dst, in_=ps[:, :])
                        else:
                            nc.scalar.copy(dst, ps[:, :])
                        eng_i += 1
                    # alternate SWDGE/HWDGE(ACT) rings so output drains
                    # overlap across chunks; last chunk rides HWDGE (0.6us
                    # completion receipt vs SWDGE's ~2us)
                    eng = nc.gpsimd if ch % 2 == 0 else nc.scalar
                    eng.dma_start(
                        out=y[b * L:(b + 1) * L, c0:c0 + chunk * C],
                        in_=yt[:, :])
                    prev_chunk = xt
    nc.compile()
    return nc


_CACHE: dict = {}


def _get_program():
    if "nc" not in _CACHE:
        _CACHE["nc"] = build_program()
        _CACHE["w"] = _build_weights()
    return _CACHE["nc"], _CACHE["w"]


def _tile_in(xs: np.ndarray) -> np.ndarray:
    """[BPC, T, C] fp32 -> [BPC*L, NBLK*C] bf16, block-tiled layout."""
    xb = xs.astype(NPBF16)
    xb = xb.reshape(BPC, NBLK, L, C).transpose(0, 2, 1, 3)
    return np.ascontiguousarray(xb).reshape(BPC * L, NBLK * C)


def _untile_out(yd: np.ndarray) -> np.ndarray:
    """[BPC*L, NBLK*C] bf16 -> [BPC, T, C] fp32."""
    yb = yd.reshape(BPC, L, NBLK, C).transpose(0, 2, 1, 3)
    return np.ascontiguousarray(yb).reshape(BPC, T, C).astype(np.float32)


def _run(x: np.ndarray, trace: bool = False):
    nc, w = _get_program()
    in_maps = [
        {"x": _tile_in(x[k * BPC:(k + 1) * BPC]), "w": w}
        for k in range(NCORES)
    ]
    res = bass_utils.run_bass_kernel_spmd(
        nc, in_maps, core_ids=list(range(NCORES)), trace=trace)
    y = np.concatenate(
        [_untile_out(r["y"]) for r in res.results], axis=0)
    return y, res


def kernel(x) -> np.ndarray:
    x = np.asarray(x, dtype=np.float32)
    assert x.shape == (B, T, C), x.shape
    y, _ = _run(x, trace=False)
    return y


# revision 5
# speedup vs baseline: 1.1000x; 1.1000x over previous
"""Debiased EMA kernel (v7): bf16 device I/O + sliding-window two-matmul.

out_block_i = P.T @ x_block_{i-1} + C.T @ x_block_i in 128-row blocks;
debias folded into block-0/1 weights.  Host casts x to bf16 and
pre-permutes to a [128, nblk*C] tiled layout so every DMA is a fully
contiguous 8KB-per-partition transfer; device computes bf16 matmuls
(1 cycle/row vs fp32's 4) into fp32 PSUM and writes bf16, host upcasts.
Halves HBM traffic both ways: ~94us DMA floor vs 187us for fp32.
Head: PE warms on a memset tile during the preamble, first input DMA
posts before the weight DMA.  Tail: deep ypool so compute never stalls
on output backlog; final chunk's output drains via four small tiles.
"""

import sys

for _p in ("/opt/trn_rl_repo", "/opt/pypackages"):
    if _p not in sys.path:
        sys.path.insert(0, _p)

import numpy as np
import ml_dtypes

import concourse.bacc as bacc
import concourse.mybir as mybir
from concourse import bass_utils
from concourse.tile import TileContext

B, T, C = 32, 4096, 512
NCORES = 8
BPC = B // NCORES
L = 128
NBLK = T // L
ALPHA = 0.9
DENOM_MIN = 1e-6

F32 = mybir.dt.float32
BF16 = mybir.dt.bfloat16
NPBF16 = ml_dtypes.bfloat16


def _build_weights() -> np.ndarray:
    a = float(np.float32(ALPHA))
    omb = 1.0 - a
    k = np.arange(L, dtype=np.float64)[:, None]
    m = np.arange(L, dtype=np.float64)[None, :]
    tri = (m - k) >= 0
    t = np.arange(2 * L, dtype=np.float64)
    d = np.maximum(1.0 - a ** (t + 1.0), DENOM_MIN)
    dec = np.where(tri, a ** np.where(tri, m - k, 0.0), 0.0)
    x0col = (k == 0)
    A0 = np.where(tri, np.where(x0col, a**m, omb * dec), 0.0) / d[:L][None, :]
    P1 = np.where(x0col, a ** (128.0 + m), omb * a ** (128.0 + m - k)) \
        / d[L:][None, :]
    C1 = omb * dec / d[L:][None, :]
    P = omb * a ** (128.0 + m - k)
    Cm = omb * dec
    w = np.concatenate([A0, P1, C1, P, Cm], axis=1)
    return np.ascontiguousarray(w.astype(NPBF16))


def build_program(bpc: int = BPC, t_len: int = T, chunk: int = 8):
    nblk = t_len // L
    nchunk = nblk // chunk
    assert nblk * L == t_len and nchunk * chunk == nblk

    nc = bacc.Bacc("TRN2", target_bir_lowering=False, debug=False)
    x = nc.dram_tensor("x", [bpc * L, nblk * C], BF16, kind="ExternalInput").ap()
    w = nc.dram_tensor("w", [L, 5 * L], BF16, kind="ExternalInput").ap()
    y = nc.dram_tensor("y", [bpc * L, nblk * C], BF16, kind="ExternalOutput").ap()

    with TileContext(nc) as tc:
        with (
            tc.tile_pool(name="wpool", bufs=1) as wpool,
            tc.tile_pool(name="xpool", bufs=6) as xpool,
            tc.tile_pool(name="ypool", bufs=8) as ypool,
            tc.tile_pool(name="tpool", bufs=4) as tpool,
            tc.tile_pool(name="psum", bufs=8, space="PSUM") as ppool,
        ):
            # Discarded matmuls on a memset tile: ramps the PE clock (HAM
            # gate) to full speed during the preamble, without waiting for
            # any DMA.
            wsrc = wpool.tile([L, C], BF16, name="warm_src")
            nc.vector.memset(wsrc[:, :], 0.0)
            warm = ppool.tile([L, C], F32, tag="ps", name="warm_ps")
            for _ in range(12):
                nc.tensor.matmul(warm[:, :], wsrc[:, 0:L], wsrc[:, :],
                                 start=True, stop=True)

            # First input chunk before the (tiny) weight load: the sync
            # ring starts streaming real data ~2us earlier.
            xt0 = xpool.tile([L, chunk * C], BF16, tag="xt", name="xt_0_0")
            nc.sync.dma_start(out=xt0[:, :], in_=x[0:L, 0:chunk * C])
            wt = wpool.tile([L, 5 * L], BF16)
            nc.sync.dma_start(out=wt[:, :], in_=w[:, :])

            A0w = wt[:, 0 * L:1 * L]
            P1w = wt[:, 1 * L:2 * L]
            C1w = wt[:, 2 * L:3 * L]
            Pw = wt[:, 3 * L:4 * L]
            Cw = wt[:, 4 * L:5 * L]

            eng_i = 0
            for b in range(bpc):
                prev_chunk = None
                for ch in range(nchunk):
                    c0 = ch * chunk * C
                    last = (b == bpc - 1 and ch == nchunk - 1)
                    if b == 0 and ch == 0:
                        xt = xt0
                    else:
                        xt = xpool.tile([L, chunk * C], BF16, tag="xt",
                                        name=f"xt_{b}_{ch}")
                        nc.sync.dma_start(
                            out=xt[:, :],
                            in_=x[b * L:(b + 1) * L, c0:c0 + chunk * C])
                    yt = None
                    if not last:
                        yt = ypool.tile([L, chunk * C], BF16, tag="yt",
                                        name=f"yt_{b}_{ch}")
                    for j in range(chunk):
                        i = ch * chunk + j
                        cur = xt[:, j * C:(j + 1) * C]
                        ps = ppool.tile([L, C], F32, tag="ps",
                                        name=f"ps_{b}_{ch}_{j}")
                        if i == 0:
                            nc.tensor.matmul(ps[:, :], A0w, cur,
                                             start=True, stop=True)
                        else:
                            prev = (xt[:, (j - 1) * C:j * C] if j > 0
                                    else prev_chunk[:, (chunk - 1) * C:])
                            pw, cw = (P1w, C1w) if i == 1 else (Pw, Cw)
                            nc.tensor.matmul(ps[:, :], pw, prev,
                                             start=True, stop=False)
                            nc.tensor.matmul(ps[:, :], cw, cur,
                                             start=False, stop=True)
                        if last:
                            # final chunk: 2-block tiles so the tail drains
                            # in small pieces right behind the copies
                            if j % 2 == 0:
                                yt = tpool.tile([L, 2 * C], BF16, tag="yt2",
                                                name=f"yt2_{j // 2}")
                            dst = yt[:, (j % 2) * C:(j % 2 + 1) * C]
                        else:
                            dst = yt[:, j * C:(j + 1) * C]
                        if eng_i % 2 == 0:
                            nc.vector.tensor_copy(out=dst, in_=ps[:, :])
                        else:
                            nc.scalar.copy(dst, ps[:, :])
                        eng_i += 1
                        if last and j % 2 == 1:
                            nc.scalar.dma_start(
                                out=y[b * L:(b + 1) * L,
                                      c0 + (j - 1) * C:c0 + (j + 1) * C],
                                in_=yt[:, :])
                    if not last:
                        # alternate SWDGE/HWDGE(ACT) rings so output drains
                        # overlap across chunks
                        eng = nc.gpsimd if ch % 2 == 0 else nc.scalar
                        eng.dma_start(
                            out=y[b * L:(b + 1) * L, c0:c0 + chunk * C],
                            in_=yt[:, :])
                    prev_chunk = xt
    nc.compile()
    return nc


_CACHE: dict = {}


def _get_program():
    if "nc" not in _CACHE:
        _CACHE["nc"] = build_program()
        _CACHE["w"] = _build_weights()
    return _CACHE["nc"], _CACHE["w"]


def _tile_in(xs: np.ndarray) -> np.ndarray:
    """[BPC, T, C] fp32 -> [BPC*L, NBLK*C] bf16, block-tiled layout."""
    xb = xs.astype(NPBF16)
    xb = xb.reshape(BPC, NBLK, L, C).transpose(0, 2, 1, 3)
    return np.ascontiguousarray(xb).reshape(BPC * L, NBLK * C)


def _untile_out(yd: np.ndarray) -> np.ndarray:
    """[BPC*L, NBLK*C] bf16 -> [BPC, T, C] fp32."""
    yb = yd.reshape(BPC, L, NBLK, C).transpose(0, 2, 1, 3)
    return np.ascontiguousarray(yb).reshape(BPC, T, C).astype(np.float32)


def _run(x: np.ndarray, trace: bool = False):
    nc, w = _get_program()
    in_maps = [
        {"x": _tile_in(x[k * BPC:(k + 1) * BPC]), "w": w}
        for k in range(NCORES)
    ]
    res = bass_utils.run_bass_kernel_spmd(
        nc, in_maps, core_ids=list(range(NCORES)), trace=trace)
    y = np.concatenate(
        [_untile_out(r["y"]) for r in res.results], axis=0)
    return y, res


def kernel(x) -> np.ndarray:
    x = np.asarray(x, dtype=np.float32)
    assert x.shape == (B, T, C), x.shape
    y, _ = _run(x, trace=False)
    return y
